# revision 1
# baseline (speedup 1.0000x reference)
"""Trainium2 Bass kernel for nn_DiceLossLayer — PE-histogram rewrite.

Data-parallel over batch: 8 masks per core on 8 cores. Per scanline y the
filled-mask count A(y,x) = #{edges e : xint_e(y) < x} is computed EXACTLY by
two tiny PE matmuls contracting over the 128 edges (partitions):

  A(8J+L) = sum_e  W[J]*CUMg[L] + W[J+1]*CUM[L]
    W[j]   = [xint < 8j]   (= [c <= 8j],  c = floor(xint)+1)
    CUM[L] = [xf < L],  xf = xint - 8*floor(xint/8),  CUMg = NOT CUM

W/CUM/CUMg are fp16 0/1 tiles with edges on partitions and (threshold, b, y)
on the free dim, built with 4x-mode tensor_scalar compares. Per y: stationary
= two adjacent 32-column slices of W, movings = CUM/CUMg 9-column slices,
PSUM out [32, 9] at quadrant offsets — 4 scanlines fill all 128 partitions.

Postprocess per PSUM tile (fp16): parity(A) via the +1024 fp16 rounding
trick, boundary = A(x+1)>A(x), mask = parity OR boundary; dice partials via
accum_out. dmap is pre-thresholded to +-1 fp16 on ACT in natural layout,
round-tripped through DRAM with a scatter DMA into the mask's (J-partition)
layout. Host combines per-tile stats into the 64 dice losses.
"""

import os

import numpy as np

os.environ.setdefault("JAX_PLATFORMS", "")

import concourse.bacc as bacc
import concourse.bass as bass
import concourse.tile as tile
from concourse import mybir
from concourse.bass_utils import run_bass_kernel_spmd

F32 = mybir.dt.float32
F16 = mybir.dt.float16
ALU = mybir.AluOpType
AF = mybir.ActivationFunctionType

N_CORES = 8
B = 8            # batches per core
NV = 128         # polygon vertices (= edges)
GRID = 256
G = 4            # batches per factor round
NJ = 32          # high digit: J = 0..31 on psum partitions (x = 8J + L)
NL = 9           # low digit grid L = 0..8 (L=8 -> next block boundary)
S0 = 56          # y-slots per quadrant, first tile of a batch (224 y)
S1 = 8           # y-slots per quadrant, second tile (32 y)
SMOOTH = 1e-6
BIGS = 10000.0   # fp16-exact sentinel for non-crossing edges


def _q_thresh() -> float:
    # largest f32 d with fl(d * 255f) <= 127f
    d = np.float32(127.0) / np.float32(255.0)
    one = np.float32(1.0)
    while np.nextafter(d, one) * np.float32(255.0) <= np.float32(127.0):
        d = np.nextafter(d, one)
    return float(d)


Q_THRESH = _q_thresh()
Q_THR_P = float(np.nextafter(np.float32(Q_THRESH), np.float32(1.0)))

_CACHE = {}


def _y_to_tile(y):
    """global y in batch (0..255) -> (tile_half, quadrant, slot)."""
    if y < 4 * S0:
        return 0, y // S0, y % S0
    y -= 4 * S0
    return 1, y // S1, y % S1


def _emit(ctx, tc, prm_d, dmap_d, q16nat_d, q16perm_d, stats_d):
    nc = tc.nc

    setup = ctx.enter_context(tc.tile_pool(name="setup", bufs=1))
    dmp = ctx.enter_context(tc.tile_pool(name="dmp", bufs=3))
    geo = ctx.enter_context(tc.tile_pool(name="geo", bufs=2))
    fact = ctx.enter_context(tc.tile_pool(name="fact", bufs=1))
    post = ctx.enter_context(tc.tile_pool(name="post", bufs=3))
    qp = ctx.enter_context(tc.tile_pool(name="qp", bufs=3))
    psum = ctx.enter_context(tc.tile_pool(name="psum", bufs=3, space="PSUM"))
    psfin = ctx.enter_context(tc.tile_pool(name="psfin", bufs=1, space="PSUM"))

    # ---------------- setup: params (host-precomputed) ----------------
    sb_prm = setup.tile([NV, 4 * B], F32)
    nc.sync.dma_start(sb_prm[:], prm_d[:])
    prm4 = sb_prm.rearrange("p (b k) -> p b k", k=4)

    # iotas
    ioty32 = setup.tile([128, GRID], F32)
    nc.gpsimd.iota(ioty32[:], pattern=[[1, GRID]], base=0, channel_multiplier=0,
                   allow_small_or_imprecise_dtypes=True)

    sb_ones9 = setup.tile([128, NL], F16)
    nc.vector.memset(sb_ones9[:], 1.0)

    # stats: per-tile T (mask sum), I2 (mask*qsgn sum); per (b, half) Qs
    NT = 2 * B
    sb_stats = setup.tile([128, 2 * NT + B], F32)
    nc.vector.memset(sb_stats[:], 0.0)

    sb_onescol = setup.tile([128, 1], F32)
    nc.vector.memset(sb_onescol[:], 1.0)
    sb_qthr = setup.tile([128, 1], F32)
    nc.vector.memset(sb_qthr[:], Q_THR_P)
    sb_zcol = setup.tile([128, 1], F32)
    nc.vector.memset(sb_zcol[:], 0.0)

    # ---------------- per-batch q path (emitted inside main loop) ----------------
    dmv = dmap_d[:]
    natap = q16nat_d[:]
    prmap = q16perm_d[:]

    def emit_qpath(b):
        sb_dm = dmp.tile([128, 2 * GRID], F32, tag="dm")
        src = bass.AP(tensor=dmv.tensor, offset=dmv.offset + b * 65536,
                      ap=[[256, 128], [128 * 256, 2], [1, 256]])
        nc.sync.dma_start(sb_dm[:], src)
        sb_qs = dmp.tile([128, 2 * GRID], F16, tag="qs")
        nc.scalar.activation(
            sb_qs[:], sb_dm[:], AF.Sign, bias=sb_qthr[:, 0:1], scale=-1.0,
            accum_out=sb_stats[:, 2 * NT + b : 2 * NT + b + 1])
        dst = bass.AP(tensor=natap.tensor, offset=natap.offset + b * 65536,
                      ap=[[256, 128], [128 * 256, 2], [1, 256]])
        nc.sync.dma_start(dst, sb_qs[:])
        for g in range(4):
            src = bass.AP(
                tensor=natap.tensor,
                offset=natap.offset + b * 65536 + g * S0 * 256,
                ap=[[256, S0], [8, NJ], [1, 8]])
            dst = bass.AP(
                tensor=prmap.tensor,
                offset=prmap.offset + (2 * b) * (128 * 448) + g * NJ * 448,
                ap=[[8, S0], [448, NJ], [1, 8]])
            nc.sync.dma_start(dst, src)
            src = bass.AP(
                tensor=natap.tensor,
                offset=natap.offset + b * 65536 + (224 + g * S1) * 256,
                ap=[[8, NJ], [256, S1], [1, 8]])
            dst = bass.AP(
                tensor=prmap.tensor,
                offset=prmap.offset + (2 * b + 1) * (128 * 448) + g * NJ * 448,
                ap=[[448, NJ], [8, S1], [1, 8]])
            nc.scalar.dma_start(dst, src)

    # ---------------- main: factor rounds + PE + postproc ----------------
    for rnd in range(B // G):
        YW = G * GRID
        xint32 = geo.tile([128, YW], F32, tag="xint32")
        xs16 = geo.tile([128, YW], F16, tag="xs16")
        xf16 = geo.tile([128, YW], F16, tag="xf16")

        for bb in range(G):
            b = rnd * G + bb
            sl = slice(bb * GRID, (bb + 1) * GRID)
            piy = prm4[:, b, 0:1]
            pjy = prm4[:, b, 1:2]
            slope = prm4[:, b, 2:3]
            beta = prm4[:, b, 3:4]

            nc.gpsimd.tensor_scalar(xint32[:, sl], ioty32[:], slope, beta,
                                    ALU.mult, ALU.add)
            c1 = geo.tile([128, GRID], F16, tag="c1")
            nc.vector.tensor_scalar(c1[:], ioty32[:], piy, None, ALU.is_gt)
            c2 = geo.tile([128, GRID], F16, tag="c2")
            nc.gpsimd.tensor_scalar(c2[:], ioty32[:], pjy, None, ALU.is_gt)
            d12 = geo.tile([128, GRID], F16, tag="d12")
            nc.vector.tensor_tensor(d12[:], c1[:], c2[:], ALU.subtract)
            crossB = geo.tile([128, GRID], F16, tag="crossB")
            nc.vector.tensor_scalar(crossB[:], d12[:], 0.0, BIGS, ALU.is_equal,
                                    ALU.mult)
            xint16 = geo.tile([128, GRID], F16, tag="xint16")
            nc.vector.tensor_scalar(xint16[:], xint32[:, sl], 300.0, -300.0,
                                    ALU.min, ALU.max)
            nc.vector.tensor_tensor(xs16[:, sl], xint16[:], crossB[:], ALU.max)

            # ch chain in fp32 from the SAME fp16-rounded xint the W compares
            # use (seam consistency); xf then exact in fp16
            xr32 = geo.tile([128, GRID], F32, tag="xr32")
            nc.scalar.activation(xr32[:], xint16[:], AF.Copy, bias=0.0,
                                 scale=1.0)
            m8 = geo.tile([128, GRID], F32, tag="m8")
            nc.gpsimd.tensor_scalar(m8[:], xr32[:], 0.125, None, ALU.mult)
            u2 = geo.tile([128, GRID], F32, tag="u2")
            nc.gpsimd.tensor_scalar(u2[:], m8[:], 8388608.0, None, ALU.add)
            r2 = geo.tile([128, GRID], F32, tag="r2")
            nc.gpsimd.tensor_scalar(r2[:], u2[:], -8388608.0, None, ALU.add)
            d2 = geo.tile([128, GRID], F32, tag="d2")
            nc.vector.tensor_tensor(d2[:], r2[:], m8[:], ALU.is_gt)
            ch = geo.tile([128, GRID], F32, tag="ch")
            nc.gpsimd.tensor_tensor(ch[:], r2[:], d2[:], ALU.subtract)
            ch8 = geo.tile([128, GRID], F32, tag="ch8")
            nc.gpsimd.tensor_scalar(ch8[:], ch[:], 8.0, None, ALU.mult)
            nc.vector.tensor_tensor(xf16[:, sl], xr32[:], ch8[:],
                                    ALU.subtract)

        # factor tiles: W (33 blocks), CUM/CUMg (9 blocks), blocks YW wide
        sb_W = fact.tile([128, 33 * YW], F16, tag="W")
        nc.vector.memset(sb_W[:, 0:YW], 0.0)
        for jj in range(1, 33):
            eng = nc.gpsimd if jj % 2 == 0 else nc.vector
            eng.tensor_scalar(sb_W[:, jj * YW : (jj + 1) * YW], xs16[:],
                              float(8 * jj), None, ALU.is_lt)
        sb_CUM = fact.tile([128, NL * YW], F16, tag="CUM")
        sb_CUMg = fact.tile([128, NL * YW], F16, tag="CUMg")
        for L in range(NL):
            nc.vector.tensor_scalar(sb_CUM[:, L * YW : (L + 1) * YW], xf16[:],
                                    float(L), None, ALU.is_lt)
            eng = nc.vector if L % 2 == 0 else nc.gpsimd
            eng.tensor_scalar(sb_CUMg[:, L * YW : (L + 1) * YW], xf16[:],
                              float(L), None, ALU.is_ge)

        wap = sb_W[:]
        cap = sb_CUM[:]
        gap = sb_CUMg[:]

        for bb in range(G):
            b = rnd * G + bb
            if b == 0:
                emit_qpath(0)
                emit_qpath(1)
            if b + 2 < B:
                emit_qpath(b + 2)
            ps_tiles = [
                psum.tile([128, 512], F32, tag="A0", name=f"A0_{b}"),
                psum.tile([128, 512], F32, tag="A1", name=f"A1_{b}"),
            ]
            # ps_tiles[0]: [128, 504] ; ps_tiles[1]: [128, 72]
            for y in range(GRID):
                th, g, i = _y_to_tile(y)
                ps = ps_tiles[th]
                off = bb * GRID + y
                statL = bass.AP(tensor=wap.tensor, offset=wap.offset + off,
                                ap=[list(wap.ap[0]), [YW, NJ]])
                statR = bass.AP(tensor=wap.tensor, offset=wap.offset + YW + off,
                                ap=[list(wap.ap[0]), [YW, NJ]])
                movC = bass.AP(tensor=cap.tensor, offset=cap.offset + off,
                               ap=[list(cap.ap[0]), [YW, NL]])
                movG = bass.AP(tensor=gap.tensor, offset=gap.offset + off,
                               ap=[list(gap.ap[0]), [YW, NL]])
                out_ap = ps[32 * g : 32 * g + 32, NL * i : NL * i + NL]
                nc.tensor.matmul(out_ap, statR, movC, start=True, stop=False,
                                 tile_position=(0, 32 * g))
                nc.tensor.matmul(out_ap, statL, movG, start=False, stop=True,
                                 tile_position=(0, 32 * g))

            # ---------------- postproc the two tiles of batch b ----------------
            qsg_big = qp.tile([128, S0 * 8], F16, tag="qsgA")
            qsg_small = qp.tile([128, S1 * 8], F16, tag="qsgB")
            qsg_tiles = [qsg_big, qsg_small]
            nc.sync.dma_start(qsg_big[:], q16perm_d[2 * b, :, 0 : S0 * 8])
            nc.sync.dma_start(qsg_small[:], q16perm_d[2 * b + 1, :, 0 : S1 * 8])
            for th, ns in ((1, S1), (0, S0)):
                t = 2 * b + th
                eA, eB = (nc.vector, nc.gpsimd) if t % 2 == 0 else (
                    nc.gpsimd, nc.vector)
                ps = ps_tiles[th]
                W_ = ns * NL  # 504 or 72
                A16 = post.tile([128, W_], F16, tag=f"A16_{th}")
                nc.scalar.activation(A16[:], ps[:, 0:W_], AF.Copy, bias=0.0,
                                     scale=1.0)
                u = post.tile([128, W_], F16, tag=f"u_{th}")
                nc.gpsimd.tensor_scalar(u[:], A16[:], 0.5, 1024.0, ALU.mult,
                                        ALU.add)
                r = post.tile([128, W_], F16, tag=f"r_{th}")
                nc.gpsimd.tensor_scalar(r[:], u[:], -1024.0, None, ALU.add)
                v = post.tile([128, W_], F16, tag=f"v_{th}")
                nc.gpsimd.tensor_scalar(v[:], r[:], -2.0, None, ALU.mult)
                dpar = post.tile([128, W_], F16, tag=f"dp_{th}")
                nc.vector.tensor_tensor(dpar[:], A16[:], v[:], ALU.add)
                par = post.tile([128, W_], F16, tag=f"par_{th}")
                nc.scalar.activation(par[:], dpar[:], AF.Abs,
                                     bias=sb_zcol[:, 0:1], scale=1.0)

                a3 = A16.rearrange("p (i l) -> p i l", l=NL)
                p3 = par.rearrange("p (i l) -> p i l", l=NL)
                bnd = post.tile([128, ns * 8], F16, tag=f"bnd_{th}")
                b3 = bnd.rearrange("p (i l) -> p i l", l=8)
                nc.vector.tensor_tensor(b3, a3[:, :, 1:9], a3[:, :, 0:8],
                                        ALU.is_gt)
                mask = post.tile([128, ns * 8], F16, tag=f"mask_{th}")
                m3 = mask.rearrange("p (i l) -> p i l", l=8)
                nc.vector.scalar_tensor_tensor(
                    m3, p3[:, :, 0:8], 0.0, b3, ALU.add, ALU.max,
                    accum_out=sb_stats[:, 2 * t : 2 * t + 1])

                qsg = qsg_tiles[th]
                i2 = qp.tile([128, ns * 8], F16, tag=f"i2_{th}")
                nc.vector.scalar_tensor_tensor(
                    i2[:], mask[:], 0.0, qsg[:], ALU.add, ALU.mult,
                    accum_out=sb_stats[:, 2 * t + 1 : 2 * t + 2])

    # ---------------- final reduction over partitions ----------------
    ncols = 2 * NT + B
    ps_fin = psfin.tile([ncols, 1], F32, tag="fin")
    nc.tensor.matmul(ps_fin[:], sb_stats[:], sb_onescol[:], start=True, stop=True)
    sb_fin = setup.tile([ncols, 1], F32)
    nc.vector.tensor_copy(sb_fin[:], ps_fin[:])
    nc.sync.dma_start(stats_d[:], sb_fin[:])


def _build():
    if "nc" in _CACHE:
        return _CACHE["nc"]
    nc = bacc.Bacc(None, target_bir_lowering=False, debug=False)
    prm_d = nc.dram_tensor("prm", [NV, 4 * B], F32, kind="ExternalInput")
    dmap_d = nc.dram_tensor("dmap", [B, GRID, GRID], F32, kind="ExternalInput")
    q16nat_d = nc.dram_tensor("q16nat", [B, GRID, GRID], F16)
    q16perm_d = nc.dram_tensor("q16perm", [2 * B, 128, 448], F16)
    stats_d = nc.dram_tensor("stats", [2 * (2 * B) + B, 1], F32,
                             kind="ExternalOutput")
    from contextlib import ExitStack

    with tile.TileContext(nc) as tc:
        with ExitStack() as ctx:
            _emit(ctx, tc, prm_d, dmap_d, q16nat_d, q16perm_d, stats_d)
    if hasattr(nc, "compile"):
        nc.compile()
    else:
        nc.finalize()
    _CACHE["nc"] = nc
    return nc


def _host_combine(stats: np.ndarray) -> np.ndarray:
    """stats: [48] -> 8 dice losses for this core's batches."""
    NT = 2 * B
    T = stats[0 : 2 * NT : 2]
    I2 = stats[1 : 2 * NT : 2]
    Qs = stats[2 * NT :]
    dices = []
    for b in range(B):
        Tb = T[2 * b] + T[2 * b + 1]
        I2b = I2[2 * b] + I2[2 * b + 1]
        Ib = 0.5 * (Tb + I2b)
        Qb = 0.5 * (Qs[b] + 65536.0)
        dices.append((2.0 * Ib + SMOOTH) / (Tb + Qb + SMOOTH))
    return np.array(dices, dtype=np.float32)


def _host_params(pts: np.ndarray) -> np.ndarray:
    """pts [B, NV, 2] -> prm [NV, 4B] fp32: (piy, pjy, slope, beta) per batch."""
    pc = np.clip(pts * np.float32(255.0), np.float32(0.0),
                 np.float32(255.0)).astype(np.float32)
    pj = np.roll(pc, 1, axis=1)
    piy, pjy = pc[:, :, 1], pj[:, :, 1]
    pix, pjx = pc[:, :, 0], pj[:, :, 0]
    d = (pjy - piy).astype(np.float32)
    d = (d + (d == 0)).astype(np.float32)
    slope = np.clip((pjx - pix) / d, -1e20, 1e20).astype(np.float32)
    beta = (pix - piy * slope).astype(np.float32)
    prm = np.stack([piy, pjy, slope, beta], axis=2)  # [B, NV, 4]
    return np.ascontiguousarray(prm.transpose(1, 0, 2).reshape(NV, 4 * B))


def kernel(points: np.ndarray, dmap: np.ndarray) -> np.ndarray:
    pts = np.asarray(points, dtype=np.float32).reshape(64, NV, 2)
    dm = np.asarray(dmap, dtype=np.float32).reshape(64, GRID, GRID)

    in_maps = []
    for r in range(N_CORES):
        sl = slice(r * B, (r + 1) * B)
        in_maps.append({
            "prm": _host_params(pts[sl]),
            "dmap": np.ascontiguousarray(dm[sl]),
        })

    nc = _build()
    res = run_bass_kernel_spmd(nc, in_maps, core_ids=list(range(N_CORES)))

    dices = []
    for r in range(N_CORES):
        s = np.asarray(res.results[r]["stats"], dtype=np.float32).reshape(-1)
        dices.append(_host_combine(s))
    dices = np.concatenate(dices).astype(np.float32)
    return np.float32(np.mean(np.float32(1.0) - dices))

